# revision 3
# baseline (speedup 1.0000x reference)
"""Trainium2 Bass kernel for nn_ClinicalEmbedding (EmbeddingBag-style ragged op).

Semantics (matches reference.py):
  flat = codes.reshape(B, L); g = renorm(W[flat])  (max_norm=1.0)
  out[b, v] = 0                       for v <  V - nv[b]
            = g[b, v - (V-nv[b])]     for V-nv[b] <= v < V-1
            = sum_{j=nv-1}^{nv*C-1} g[b, j]   for v = V-1

Strategy: vocab-sharded scan (no gathers at all -- random row gather via
SWDGE indirect DMA measures ~253 ns/row on this HW, and the fast custom
dma_gather ucode is unavailable). Core k owns vocab rows [k*SH, (k+1)*SH):
  * streams its shard (partition-major, fully contiguous DMA),
  * renorms every row (DVE/ACT),
  * writes the renormed rows back out (gdump) for the host to pick the
    "single" outputs from,
  * accumulates bag partial sums for ALL 256 patients with one fp32r
    matmul per 128-row block: psum[E, 256] += g_blk.T @ mult_blk, where
    mult_blk[p, s] = multiplicity of shard row (blk*128+p) in patient s's
    last-visit bag.
Host side: sums the 8 partial-sum outputs (reduce-unshard), picks singles
rows out of the gdumps, and assembles the full [B, V, E] output.
"""

import os

import numpy as np

import concourse.bacc as bacc
import concourse.bass as bass
import concourse.mybir as mybir
import concourse.tile as tile
from concourse.bass_utils import run_bass_kernel_spmd

P = 128
N_CORES = 8
B, V, C = 256, 50, 32
L = V * C
VOCAB, E = 100000, 128
SH = 12544              # vocab rows per core (98 blocks of 128; 8*SH >= VOCAB)
NBLK = SH // P          # 98
S = B                   # reduction slots = all patients
TB = 14                 # blocks per processing tile
NT = NBLK // TB         # 7 tiles

LAST_RESULTS = None     # test harness reads profiling info from here


def _prepare(W, codes, nv):
    """Host-side structure. codes: [B, L] int64/int32, nv: [B] int64."""
    flat = codes.reshape(B, L)
    # ---- per-core shard, partition-major [P, NBLK*E] ----
    Wpad = np.zeros((N_CORES * SH, E), np.float32)
    Wpad[:VOCAB] = W
    # WshT[k][p, blk*E+e] = Wpad[k*SH + blk*128 + p, e]
    WshT = (
        Wpad.reshape(N_CORES, NBLK, P, E)
        .transpose(0, 2, 1, 3)
        .reshape(N_CORES, P, NBLK * E)
    )
    WshT = np.ascontiguousarray(WshT)

    # ---- bag draws -> mult[k][p, blk*S + s] ----
    bag_vals, bag_slot = [], []
    for b in range(B):
        n = int(nv[b])
        vals = flat[b, n - 1 : n * C]
        bag_vals.append(vals)
        bag_slot.append(np.full(len(vals), b, np.int64))
    u = np.concatenate(bag_vals).astype(np.int64)
    s = np.concatenate(bag_slot)
    k = u // SH
    loc = u - k * SH
    p, blk = loc % P, loc // P
    lin = ((k * P + p) * NBLK + blk) * S + s
    mult = np.bincount(lin, minlength=N_CORES * P * NBLK * S).astype(np.float32)
    mult = mult.reshape(N_CORES, P, NBLK * S)

    # ---- singles map: (b, v) -> (k, p, blk) ----
    sb, sv, su = [], [], []
    for b in range(B):
        n = int(nv[b])
        if n <= 1:
            continue
        v = np.arange(V - n, V - 1)
        sb.append(np.full(n - 1, b, np.int64))
        sv.append(v)
        su.append(flat[b, v - (V - n)])
    sb = np.concatenate(sb)
    sv = np.concatenate(sv)
    su = np.concatenate(su).astype(np.int64)
    sk = su // SH
    sloc = su - sk * SH
    sp, sblk = sloc % P, sloc // P

    return dict(WshT=WshT, mult=mult, sb=sb, sv=sv, sk=sk, sp=sp, sblk=sblk)


def _build():
    """Emit the Bass/Tile program (shared across all 8 cores)."""
    f32 = mybir.dt.float32
    f32r = mybir.dt.float32r

    nc = bacc.Bacc("TRN2", num_devices=N_CORES, debug=False)
    wsh_d = nc.dram_tensor("wsh", [P, NBLK * E], f32, kind="ExternalInput")
    mult_d = nc.dram_tensor("mult", [P, NBLK * S], f32, kind="ExternalInput")
    gdump_d = nc.dram_tensor("gdump", [P, NBLK * E], f32, kind="ExternalOutput")
    part_d = nc.dram_tensor("part", [P, S], f32, kind="ExternalOutput")

    with tile.TileContext(nc) as tc:
        with (
            tc.tile_pool(name="g", bufs=3) as gpool,
            tc.tile_pool(name="m", bufs=2) as mpool,
            tc.tile_pool(name="sq", bufs=2) as sqpool,
            tc.tile_pool(name="gp", bufs=3) as gppool,
            tc.tile_pool(name="sm", bufs=2) as smpool,
            tc.tile_pool(name="ps", bufs=1, space="PSUM") as pspool,
        ):
            psum = pspool.tile([P, S], f32)

            # zero bias tile written by DVE so ACT sqrt waits only on DVE
            zbias = smpool.tile([P, 1], f32, tag="zbias", bufs=1)
            nc.vector.memset(zbias[:], 0.0)

            for t in range(NT):
                b0 = t * TB
                g = gpool.tile([P, TB * E], f32, tag="g")
                nc.sync.dma_start(g[:], wsh_d[:, b0 * E : (b0 + TB) * E])
                m = mpool.tile([P, TB * S], f32, tag="m")
                nc.sync.dma_start(m[:], mult_d[:, b0 * S : (b0 + TB) * S])

                sq = sqpool.tile([P, TB * E], f32, tag="sq")
                nc.vector.tensor_mul(sq[:], g[:], g[:])
                n2 = smpool.tile([P, TB], f32, tag="n2")
                nc.vector.tensor_reduce(
                    n2[:], sq[:].rearrange("p (c e) -> p c e", e=E),
                    axis=mybir.AxisListType.X, op=mybir.AluOpType.add,
                )
                nc.vector.tensor_scalar_max(n2[:], n2[:], 1.0)
                sr = smpool.tile([P, TB], f32, tag="sr")
                nc.scalar.activation(
                    sr[:], n2[:], mybir.ActivationFunctionType.Sqrt, bias=zbias[:]
                )
                rr = smpool.tile([P, TB], f32, tag="rr")
                nc.vector.reciprocal(rr[:], sr[:])
                gp = gppool.tile([P, TB * E], f32, tag="gp")
                nc.vector.tensor_tensor(
                    out=gp[:].rearrange("p (c e) -> p c e", e=E),
                    in0=g[:].rearrange("p (c e) -> p c e", e=E),
                    in1=rr[:].to_broadcast([P, TB, E]),
                    op=mybir.AluOpType.mult,
                )
                nc.scalar.dma_start(
                    gdump_d[:, b0 * E : (b0 + TB) * E], gp[:]
                )
                for j in range(TB):
                    blk = b0 + j
                    nc.tensor.matmul(
                        out=psum[:, :],
                        lhsT=gp[:, j * E : (j + 1) * E],
                        rhs=m[:, j * S : (j + 1) * S],
                        start=(blk == 0),
                        stop=(blk == NBLK - 1),
                    )

            outS = smpool.tile([P, S], f32, tag="outS", bufs=1)
            nc.vector.tensor_copy(outS[:], psum[:])
            nc.sync.dma_start(part_d[:], outS[:])

    nc.compile()
    return nc


def _assemble(prep, gdumps, parts):
    """gdumps: [N_CORES][P, NBLK*E]; parts: [N_CORES][P(E), S]."""
    full = np.empty((B, V, E), np.float32)
    # zero rows: every core-7 row past VOCAB is an exact-zero renormed row
    zrow = gdumps[N_CORES - 1][P - 1, (NBLK - 1) * E : NBLK * E]
    full[:, :, :] = zrow[None, None, :]
    # bag row v = V-1: sum of per-shard partials (reduce-unshard)
    bag = parts.sum(axis=0).T           # [S, E]
    full[:, V - 1, :] = bag
    # singles
    gd = gdumps.reshape(N_CORES, P, NBLK, E)
    full[prep["sb"], prep["sv"]] = gd[prep["sk"], prep["sp"], prep["sblk"]]
    return full


def kernel(**inputs) -> np.ndarray:
    global LAST_RESULTS
    W = np.ascontiguousarray(np.asarray(inputs["W"], dtype=np.float32))
    codes_in = np.asarray(inputs["codes"])
    nv = np.asarray(inputs["n_visits"]).astype(np.int64)

    codes = np.ascontiguousarray(codes_in.reshape(B, L).astype(np.int64))
    prep = _prepare(W, codes, nv)
    nc = _build()

    in_maps = [
        {"wsh": prep["WshT"][k], "mult": prep["mult"][k]} for k in range(N_CORES)
    ]
    trace = bool(int(os.environ.get("KERNEL_TRACE", "0")))
    res = run_bass_kernel_spmd(
        nc, in_maps, core_ids=list(range(N_CORES)), trace=trace
    )
    LAST_RESULTS = res

    gdumps = np.stack([res.results[k]["gdump"] for k in range(N_CORES)])
    parts = np.stack([res.results[k]["part"] for k in range(N_CORES)])
    return _assemble(prep, gdumps, parts)
